# revision 9
# baseline (speedup 1.0000x reference)
"""Per-task adapter (MoE routing) on 8 TRN2 NeuronCores.

Strategy: expert-parallel with host-side routing. Each core owns 2 of the
16 tasks. The host sorts samples by task (the "all-to-all dispatch" is free
because kernel() receives full inputs), pads each task's rows to a common
capacity, and hands each core exactly the x-rows routed to its tasks plus
its 2 tasks' adapter weights. On device: dense fp8 matmuls
(down-proj -> SiLU -> up-proj) in transposed layout, no collectives.
The host applies the inverse permutation, residual add (f32-exact) and
up-bias while reassembling.

fp8 scheme: weights are scaled by 256 on the host (values land well inside
TRN e4m3's +-240 normal range), the SiLU activation folds the 1/256 back in
via its input scale, and the up-projection epilogue multiplies by 1/256.
x (|x| < ~5) and act (|act| < ~4) fit e4m3 directly.

All DRAM parameters are laid out partition-major by the host so every
load/store is one large linear DMA (128 descriptors of contiguous rows).
"""

import os
import sys

import numpy as np

sys.path.insert(0, "/opt/trn_rl_repo")

D = 4096          # model dim
H = 256           # adapter bottleneck dim
T = 16            # number of tasks
NCORES = 8
TPC = T // NCORES  # tasks per core = 2
KD = D // 128      # 32 k-tiles over model dim
KH = H // 128      # 2 k-tiles over bottleneck dim
CK_MAX = 288       # max rows per matmul chunk (SBUF/PSUM limits)
WSCALE = 256.0     # host-side fp8 weight scale

# "fp8": fp8 x/weights/out, residual on host (fast path).
# "bf16": bf16 weights/matmul, f32 x and residual on device (precise path).
MODE = os.environ.get("KERNEL_MODE", "fp8")

_BUILD_CACHE = {}
LAST_RESULT = None


def _build_fp8(nch: int, ck: int):
    """fp8 graph: x,wd,wu,out all fp8(e4m3); psum f32; silu on ACT."""
    import concourse.bass as bass  # noqa: F401
    import concourse.bacc as bacc
    import concourse.tile as tile
    from concourse import mybir

    f32 = mybir.dt.float32
    fp8 = mybir.dt.float8e4
    Silu = mybir.ActivationFunctionType.Silu
    Copy = mybir.ActivationFunctionType.Copy

    nc = bacc.Bacc(
        "TRN2", target_bir_lowering=False, debug=False, num_devices=NCORES
    )

    xt_d = nc.declare_dram_parameter(
        "xt", [TPC, nch, 128, KD * ck], fp8, isOutput=False
    )
    # wd and wu packed per task: [KH*KD*128 (wd) | KH*D (wu)] per task
    wtask = KH * KD * 128 + KH * D
    w_d = nc.declare_dram_parameter("w", [128, TPC * wtask], fp8, isOutput=False)
    bd_d = nc.declare_dram_parameter("bd", [128, TPC * KH], f32, isOutput=False)
    out_d = nc.declare_dram_parameter(
        "out", [TPC, nch, 128, KD * ck], fp8, isOutput=True
    )

    with tile.TileContext(nc) as tc:
        with (
            tc.tile_pool(name="wpool", bufs=1) as wpool,
            tc.tile_pool(name="xp", bufs=2) as xpool,
            tc.tile_pool(name="act", bufs=2) as apool,
            tc.tile_pool(name="ot", bufs=2) as opool,
            tc.tile_pool(name="psum", bufs=2, space="PSUM") as pspool,
        ):
            w_sb = wpool.tile([128, TPC * wtask], fp8, name="w_sb")
            bd_sb = wpool.tile([128, TPC * KH], f32, name="bd_sb")

            def wd_ap(t, k, h):  # lhsT [128, 128] for down matmul
                base = t * wtask + (h * KD + k) * 128
                return w_sb[:, base: base + 128]

            def wu_ap(t, k, m):  # lhsT [128, 128] for up matmul
                base = t * wtask + KH * KD * 128 + k * D + m * 128
                return w_sb[:, base: base + 128]

            # PE warm-up: dummy matmuls on a zeroed scratch tile keep the
            # HAM activity window busy while the first DMAs land, so real
            # matmuls start at 2.4 GHz instead of 1.2 GHz.
            wtile = wpool.tile([128, 128], fp8, name="wtile")
            wps = pspool.tile([128, 128], f32, name="wps", tag="d0", bufs=2)
            nc.vector.memset(wtile[:], 0)
            for i in range(18):
                nc.tensor.matmul(
                    wps[:], wtile[:], wtile[:], start=True, stop=True
                )

            xall = {}
            # x on sync, weights on scalar: parallel DMA issue; x first and
            # wd(t0,h0) as its own small DMA so down-matmuls start early.
            wslab = KD * 128
            for t in range(TPC):
                xall[t] = [
                    xpool.tile([128, KD * ck], fp8, name=f"xall{t}_{c}",
                               tag=f"xall{c % 2}")
                    for c in range(nch)
                ]
                nc.sync.dma_start(xall[t][0][:], xt_d[t, 0])
                for h in range(KH):
                    base = t * wtask + h * wslab
                    nc.scalar.dma_start(
                        w_sb[:, base: base + wslab],
                        w_d[:, base: base + wslab],
                    )
                base = t * wtask + KH * wslab
                for q in range(2):
                    nc.scalar.dma_start(
                        w_sb[:, base + q * D: base + (q + 1) * D],
                        w_d[:, base + q * D: base + (q + 1) * D],
                    )
                if t == TPC - 1:
                    nc.scalar.dma_start(bd_sb[:], bd_d[:])
                for c in range(1, nch):
                    nc.sync.dma_start(xall[t][c][:], xt_d[t, c])

            # Per (t, c): down then up. PE executes in program order, so this
            # keeps each phase's data dependencies as early as possible.
            for t in range(TPC):
                for c in range(nch):
                    xa = xall[t][c]
                    act = [
                        apool.tile([128, ck], fp8, name=f"act{h}", tag=f"act{h}")
                        for h in range(KH)
                    ]
                    for h in range(KH):
                        ps = pspool.tile([128, ck], f32, name=f"psd{h}", tag=f"d{h}")
                        for k in range(KD):
                            nc.tensor.matmul(
                                ps[:],
                                wd_ap(t, k, h),
                                xa[:, k * ck:(k + 1) * ck],
                                start=(k == 0),
                                stop=(k == KD - 1),
                            )
                        # act = silu(psum/WSCALE + bd)
                        nc.scalar.activation(
                            act[h][:], ps[:], Silu,
                            bias=bd_sb[:, t * KH + h: t * KH + h + 1],
                            scale=1.0 / WSCALE,
                        )

                    # up-projection; epilogue out = psum/WSCALE (residual+bias
                    # are applied on the host). 2 m-tiles per psum bank; the
                    # scale op alternates between DVE and ACT; stores per
                    # eighth overlap with the remaining matmuls.
                    oall = opool.tile([128, KD * ck], fp8, name="oall", tag="oall")
                    for m2 in range(KD // 2):
                        psu = pspool.tile(
                            [128, 2 * ck], f32, name="psu", tag="u", bufs=4
                        )
                        for half in range(2):
                            m = 2 * m2 + half
                            for k in range(KH):
                                nc.tensor.matmul(
                                    psu[:, half * ck:(half + 1) * ck],
                                    wu_ap(t, k, m),
                                    act[k][:],
                                    start=(k == 0),
                                    stop=(k == KH - 1),
                                )
                        osl = oall[:, 2 * m2 * ck:(2 * m2 + 2) * ck]
                        if m2 % 2 == 0:
                            nc.vector.tensor_scalar_mul(osl, psu[:], 1.0 / WSCALE)
                        else:
                            nc.scalar.activation(osl, psu[:], Copy, scale=1.0 / WSCALE)
                        if m2 % (KD // 16) == KD // 16 - 1:
                            q = (m2 + 1) // (KD // 16) - 1
                            qsz = KD * ck // 8
                            nc.sync.dma_start(
                                out_d[t, c, :, q * qsz:(q + 1) * qsz],
                                oall[:, q * qsz:(q + 1) * qsz],
                            )

    nc.compile()
    return nc


def _build_bf16(nch: int, ck: int):
    """Precise graph: bf16 matmuls, f32 x + on-device f32 residual."""
    import concourse.bass as bass  # noqa: F401
    import concourse.bacc as bacc
    import concourse.tile as tile
    from concourse import mybir

    f32 = mybir.dt.float32
    bf16 = mybir.dt.bfloat16
    Silu = mybir.ActivationFunctionType.Silu
    add = mybir.AluOpType.add

    nc = bacc.Bacc(
        "TRN2", target_bir_lowering=False, debug=False, num_devices=NCORES
    )

    xt_d = nc.declare_dram_parameter(
        "xt", [TPC, nch, 128, KD * ck], f32, isOutput=False
    )
    wd_d = nc.declare_dram_parameter(
        "wd", [128, TPC * KH * KD * 128], bf16, isOutput=False
    )
    wu_d = nc.declare_dram_parameter(
        "wu", [128, TPC * KH * D], bf16, isOutput=False
    )
    bd_d = nc.declare_dram_parameter("bd", [128, TPC * KH], f32, isOutput=False)
    bu_d = nc.declare_dram_parameter("bu", [128, TPC * KD], f32, isOutput=False)
    out_d = nc.declare_dram_parameter(
        "out", [TPC, nch, 128, KD * ck], f32, isOutput=True
    )

    with tile.TileContext(nc) as tc:
        with (
            tc.tile_pool(name="wpool", bufs=1) as wpool,
            tc.tile_pool(name="xf", bufs=2) as xfpool,
            tc.tile_pool(name="xb", bufs=2) as xbpool,
            tc.tile_pool(name="act", bufs=2) as apool,
            tc.tile_pool(name="ot", bufs=2) as opool,
            tc.tile_pool(name="psum", bufs=2, space="PSUM") as pspool,
        ):
            wd_sb = wpool.tile([128, TPC * KH * KD * 128], bf16, name="wd_sb")
            wu_sb = wpool.tile([128, TPC * KH * D], bf16, name="wu_sb")
            bd_sb = wpool.tile([128, TPC * KH], f32, name="bd_sb")
            bu_sb = wpool.tile([128, TPC * KD], f32, name="bu_sb")

            def wd_ap(t, k, h):
                base = ((t * KH + h) * KD + k) * 128
                return wd_sb[:, base: base + 128]

            def wu_ap(t, k, m):
                base = (t * KH + k) * D + m * 128
                return wu_sb[:, base: base + 128]

            xall = {}
            for t in range(TPC):
                xall[t] = [
                    xfpool.tile([128, KD * ck], f32, name=f"xall{t}_{c}",
                                tag=f"xall{c % 2}")
                    for c in range(nch)
                ]
                nc.sync.dma_start(xall[t][0][:], xt_d[t, 0])
                wslab = KD * 128
                for h in range(KH):
                    base = (t * KH + h) * wslab
                    nc.sync.dma_start(
                        wd_sb[:, base: base + wslab],
                        wd_d[:, base: base + wslab],
                    )
                for k in range(KH):
                    base = (t * KH + k) * D
                    nc.sync.dma_start(
                        wu_sb[:, base: base + D],
                        wu_d[:, base: base + D],
                    )
                if t == 0:
                    nc.sync.dma_start(bd_sb[:], bd_d[:])
                    nc.sync.dma_start(bu_sb[:], bu_d[:])
                for c in range(1, nch):
                    nc.sync.dma_start(xall[t][c][:], xt_d[t, c])

            for t in range(TPC):
                for c in range(nch):
                    xa = xall[t][c]
                    xb = xbpool.tile([128, KD * ck], bf16, name="xb", tag="xb")
                    for k in range(KD):
                        nc.vector.tensor_copy(
                            xb[:, k * ck:(k + 1) * ck],
                            xa[:, k * ck:(k + 1) * ck],
                        )
                    act = [
                        apool.tile([128, ck], bf16, name=f"act{h}", tag=f"act{h}")
                        for h in range(KH)
                    ]
                    for h in range(KH):
                        ps = pspool.tile([128, ck], f32, name=f"psd{h}", tag=f"d{h}")
                        for k in range(KD):
                            nc.tensor.matmul(
                                ps[:],
                                wd_ap(t, k, h),
                                xb[:, k * ck:(k + 1) * ck],
                                start=(k == 0),
                                stop=(k == KD - 1),
                            )
                        nc.scalar.activation(
                            act[h][:], ps[:], Silu,
                            bias=bd_sb[:, t * KH + h: t * KH + h + 1],
                            scale=1.0,
                        )
                    oall = opool.tile([128, KD * ck], f32, name="oall", tag="oall")
                    for m in range(KD):
                        psu = pspool.tile([128, ck], f32, name="psu", tag="u", bufs=3)
                        for k in range(KH):
                            nc.tensor.matmul(
                                psu[:],
                                wu_ap(t, k, m),
                                act[k][:],
                                start=(k == 0),
                                stop=(k == KH - 1),
                            )
                        nc.vector.scalar_tensor_tensor(
                            oall[:, m * ck:(m + 1) * ck], psu[:],
                            bu_sb[:, t * KD + m: t * KD + m + 1],
                            xa[:, m * ck:(m + 1) * ck],
                            op0=add, op1=add,
                        )
                    nc.sync.dma_start(out_d[t, c], oall[:])

    nc.compile()
    return nc


def kernel(x, task_id, Wd, bd, Wu, bu):
    global LAST_RESULT
    from concourse.bass_utils import run_bass_kernel_spmd
    from concourse import mybir

    fp8_mode = MODE == "fp8"
    bf16_np = mybir.dt.np(mybir.dt.bfloat16)
    fp8_np = mybir.dt.np(mybir.dt.float8e4)
    w_np = fp8_np if fp8_mode else bf16_np

    x = np.ascontiguousarray(np.asarray(x, dtype=np.float32))
    tid = np.asarray(task_id).astype(np.int64)
    Wd = np.asarray(Wd, dtype=np.float32)
    bd = np.asarray(bd, dtype=np.float32)
    Wu = np.asarray(Wu, dtype=np.float32)
    bu = np.asarray(bu, dtype=np.float32)
    B = x.shape[0]

    # --- host-side routing (the all-to-all dispatch) ---
    order = np.argsort(tid, kind="stable")
    counts = np.bincount(tid, minlength=T)
    starts = np.concatenate([[0], np.cumsum(counts)])[:T]
    cap = max(int(counts.max()), 1)
    nch = -(-cap // CK_MAX)
    ck = -(-(-(-cap // nch)) // 8) * 8  # ceil(cap/nch) rounded up to x8
    rows_per_task = nch * ck

    x_sorted = x[order]
    wscale = WSCALE if fp8_mode else 1.0

    in_maps = []
    for g in range(NCORES):
        xpad = np.zeros((TPC, rows_per_task, D), np.float32)
        for t in range(TPC):
            j = TPC * g + t
            n = counts[j]
            xpad[t, :n] = x_sorted[starts[j]: starts[j] + n]
        # [TPC, nch, 128(p), KD*ck] with col k*ck+j <-> (row j, d-tile k)
        xt_in = np.ascontiguousarray(
            xpad.reshape(TPC, nch, ck, KD, 128).transpose(0, 1, 4, 3, 2)
        ).reshape(TPC, nch, 128, KD * ck)
        sl = slice(TPC * g, TPC * g + TPC)
        # wd: [128p, TPC, KH(h), KD(k), 128(col within h-block)]
        wd_in = (
            (Wd[sl] * wscale).reshape(TPC, KD, 128, KH, 128)
            .transpose(2, 0, 3, 1, 4)
        ).reshape(128, TPC, KH * KD * 128)
        wu_in = (
            (Wu[sl] * wscale).reshape(TPC, KH, 128, D).transpose(2, 0, 1, 3)
        ).reshape(128, TPC, KH * D)
        if fp8_mode:
            w_in = np.concatenate([wd_in, wu_in], axis=2)  # [128, TPC, wtask]
            m = {
                "xt": xt_in.astype(fp8_np),
                "w": np.ascontiguousarray(
                    w_in.reshape(128, -1)
                ).astype(fp8_np),
                "bd": np.ascontiguousarray(bd[sl].reshape(TPC * KH, 128).T),
            }
        else:
            m = {
                "xt": xt_in,
                "wd": np.ascontiguousarray(wd_in.reshape(128, -1)).astype(w_np),
                "wu": np.ascontiguousarray(wu_in.reshape(128, -1)).astype(w_np),
                "bd": np.ascontiguousarray(bd[sl].reshape(TPC * KH, 128).T),
                "bu": np.ascontiguousarray(bu[sl].reshape(TPC * KD, 128).T),
            }
        in_maps.append(m)

    key = (MODE, nch, ck)
    if key not in _BUILD_CACHE:
        build = _build_fp8 if fp8_mode else _build_bf16
        _BUILD_CACHE[key] = build(nch, ck)
    nc = _BUILD_CACHE[key]

    res = run_bass_kernel_spmd(nc, in_maps, core_ids=list(range(NCORES)))
    LAST_RESULT = res

    # --- gather / unshard (inverse permutation; fp8 mode: +x +bu here) ---
    out_full = np.empty((B, D), np.float32)
    for g in range(NCORES):
        o = np.asarray(res.results[g]["out"]).astype(np.float32)
        o = o.reshape(TPC, nch, 128, KD, ck)
        o = o.transpose(0, 1, 4, 3, 2).reshape(TPC, rows_per_task, D)
        for t in range(TPC):
            j = TPC * g + t
            n = counts[j]
            rows = order[starts[j]: starts[j] + n]
            if fp8_mode:
                out_full[rows] = x[rows] + o[t, :n] + bu[j][None, :]
            else:
                out_full[rows] = o[t, :n]
    return out_full


# revision 10
# speedup vs baseline: 1.0571x; 1.0571x over previous
"""Per-task adapter (MoE routing) on 8 TRN2 NeuronCores.

Strategy: expert-parallel with host-side routing. Each core owns 2 of the
16 tasks. The host sorts samples by task (the "all-to-all dispatch" is free
because kernel() receives full inputs), pads each task's rows to a common
capacity, and hands each core exactly the x-rows routed to its tasks plus
its 2 tasks' adapter weights. On device: dense fp8 matmuls
(down-proj -> SiLU -> up-proj) in transposed layout, no collectives.
The host applies the inverse permutation, residual add (f32-exact) and
up-bias while reassembling.

fp8 scheme: weights are scaled by 256 on the host (values land well inside
TRN e4m3's +-240 normal range), the SiLU activation folds the 1/256 back in
via its input scale, and the up-projection epilogue multiplies by 1/256.
x (|x| < ~5) and act (|act| < ~4) fit e4m3 directly.

All DRAM parameters are laid out partition-major by the host so every
load/store is one large linear DMA (128 descriptors of contiguous rows).
"""

import os
import sys

import numpy as np

sys.path.insert(0, "/opt/trn_rl_repo")

D = 4096          # model dim
H = 256           # adapter bottleneck dim
T = 16            # number of tasks
NCORES = 8
TPC = T // NCORES  # tasks per core = 2
KD = D // 128      # 32 k-tiles over model dim
KH = H // 128      # 2 k-tiles over bottleneck dim
CK_MAX = 288       # max rows per matmul chunk (SBUF/PSUM limits)
WSCALE = 256.0     # host-side fp8 weight scale

# "fp8": fp8 x/weights/out, residual on host (fast path).
# "bf16": bf16 weights/matmul, f32 x and residual on device (precise path).
MODE = os.environ.get("KERNEL_MODE", "fp8")

_BUILD_CACHE = {}
LAST_RESULT = None


def _build_fp8(nch: int, ck: int):
    """fp8 graph: x,wd,wu,out all fp8(e4m3); psum f32; silu on ACT."""
    import concourse.bass as bass  # noqa: F401
    import concourse.bacc as bacc
    import concourse.tile as tile
    from concourse import mybir

    f32 = mybir.dt.float32
    fp8 = mybir.dt.float8e4
    Silu = mybir.ActivationFunctionType.Silu
    Copy = mybir.ActivationFunctionType.Copy

    nc = bacc.Bacc(
        "TRN2", target_bir_lowering=False, debug=False, num_devices=NCORES
    )

    xt_d = nc.declare_dram_parameter(
        "xt", [TPC, nch, 128, KD * ck], fp8, isOutput=False
    )
    # wd and wu packed per task: [KH*KD*128 (wd) | KH*D (wu)] per task
    wtask = KH * KD * 128 + KH * D
    w_d = nc.declare_dram_parameter("w", [128, TPC * wtask], fp8, isOutput=False)
    bd_d = nc.declare_dram_parameter("bd", [128, TPC * KH], f32, isOutput=False)
    out_d = nc.declare_dram_parameter(
        "out", [TPC, nch, 128, KD * ck], fp8, isOutput=True
    )

    with tile.TileContext(nc) as tc:
        with (
            tc.tile_pool(name="wpool", bufs=1) as wpool,
            tc.tile_pool(name="xp", bufs=2) as xpool,
            tc.tile_pool(name="act", bufs=2) as apool,
            tc.tile_pool(name="ot", bufs=2) as opool,
            tc.tile_pool(name="psum", bufs=2, space="PSUM") as pspool,
        ):
            w_sb = wpool.tile([128, TPC * wtask], fp8, name="w_sb")
            bd_sb = wpool.tile([128, TPC * KH], f32, name="bd_sb")

            def wd_ap(t, k, h):  # lhsT [128, 128] for down matmul
                base = t * wtask + (h * KD + k) * 128
                return w_sb[:, base: base + 128]

            def wu_ap(t, k, m):  # lhsT [128, 128] for up matmul
                base = t * wtask + KH * KD * 128 + k * D + m * 128
                return w_sb[:, base: base + 128]

            # PE warm-up: dummy matmuls on a zeroed scratch tile keep the
            # HAM activity window busy while the first DMAs land, so real
            # matmuls start at 2.4 GHz instead of 1.2 GHz.
            wtile = wpool.tile([128, 128], fp8, name="wtile")
            wps = pspool.tile([128, 128], f32, name="wps", tag="d0", bufs=2)
            nc.vector.memset(wtile[:], 0)
            for i in range(18):
                nc.tensor.matmul(
                    wps[:], wtile[:], wtile[:], start=True, stop=True
                )

            xall = {}
            # x on sync, weights on scalar: parallel DMA issue; x first and
            # wd(t0,h0) as its own small DMA so down-matmuls start early.
            wslab = KD * 128
            for t in range(TPC):
                xall[t] = [
                    xpool.tile([128, KD * ck], fp8, name=f"xall{t}_{c}",
                               tag=f"xall{c % 2}")
                    for c in range(nch)
                ]
                if t == 0:
                    nc.sync.dma_start(bd_sb[:], bd_d[:])
                nc.sync.dma_start(xall[t][0][:], xt_d[t, 0])
                for h in range(KH):
                    base = t * wtask + h * wslab
                    nc.scalar.dma_start(
                        w_sb[:, base: base + wslab],
                        w_d[:, base: base + wslab],
                    )
                base = t * wtask + KH * wslab
                for q in range(2):
                    nc.scalar.dma_start(
                        w_sb[:, base + q * D: base + (q + 1) * D],
                        w_d[:, base + q * D: base + (q + 1) * D],
                    )
                for c in range(1, nch):
                    nc.sync.dma_start(xall[t][c][:], xt_d[t, c])

            # Per (t, c): down then up. PE executes in program order, so this
            # keeps each phase's data dependencies as early as possible.
            for t in range(TPC):
                for c in range(nch):
                    xa = xall[t][c]
                    act = [
                        apool.tile([128, ck], fp8, name=f"act{h}", tag=f"act{h}")
                        for h in range(KH)
                    ]
                    for h in range(KH):
                        ps = pspool.tile([128, ck], f32, name=f"psd{h}", tag=f"d{h}")
                        for k in range(KD):
                            nc.tensor.matmul(
                                ps[:],
                                wd_ap(t, k, h),
                                xa[:, k * ck:(k + 1) * ck],
                                start=(k == 0),
                                stop=(k == KD - 1),
                            )
                        # act = silu(psum/WSCALE + bd)
                        nc.scalar.activation(
                            act[h][:], ps[:], Silu,
                            bias=bd_sb[:, t * KH + h: t * KH + h + 1],
                            scale=1.0 / WSCALE,
                        )

                    # up-projection; epilogue out = psum/WSCALE (residual+bias
                    # are applied on the host). 2 m-tiles per psum bank; the
                    # scale op alternates between DVE and ACT; stores per
                    # eighth overlap with the remaining matmuls.
                    oall = opool.tile([128, KD * ck], fp8, name="oall", tag="oall")
                    for m2 in range(KD // 2):
                        psu = pspool.tile(
                            [128, 2 * ck], f32, name="psu", tag="u", bufs=4
                        )
                        for half in range(2):
                            m = 2 * m2 + half
                            for k in range(KH):
                                nc.tensor.matmul(
                                    psu[:, half * ck:(half + 1) * ck],
                                    wu_ap(t, k, m),
                                    act[k][:],
                                    start=(k == 0),
                                    stop=(k == KH - 1),
                                )
                        osl = oall[:, 2 * m2 * ck:(2 * m2 + 2) * ck]
                        if m2 % 2 == 0:
                            nc.vector.tensor_scalar_mul(osl, psu[:], 1.0 / WSCALE)
                        else:
                            nc.scalar.activation(osl, psu[:], Copy, scale=1.0 / WSCALE)
                        if m2 % (KD // 16) == KD // 16 - 1:
                            q = (m2 + 1) // (KD // 16) - 1
                            qsz = KD * ck // 8
                            nc.sync.dma_start(
                                out_d[t, c, :, q * qsz:(q + 1) * qsz],
                                oall[:, q * qsz:(q + 1) * qsz],
                            )

    nc.compile()
    return nc


def _build_bf16(nch: int, ck: int):
    """Precise graph: bf16 matmuls, f32 x + on-device f32 residual."""
    import concourse.bass as bass  # noqa: F401
    import concourse.bacc as bacc
    import concourse.tile as tile
    from concourse import mybir

    f32 = mybir.dt.float32
    bf16 = mybir.dt.bfloat16
    Silu = mybir.ActivationFunctionType.Silu
    add = mybir.AluOpType.add

    nc = bacc.Bacc(
        "TRN2", target_bir_lowering=False, debug=False, num_devices=NCORES
    )

    xt_d = nc.declare_dram_parameter(
        "xt", [TPC, nch, 128, KD * ck], f32, isOutput=False
    )
    wd_d = nc.declare_dram_parameter(
        "wd", [128, TPC * KH * KD * 128], bf16, isOutput=False
    )
    wu_d = nc.declare_dram_parameter(
        "wu", [128, TPC * KH * D], bf16, isOutput=False
    )
    bd_d = nc.declare_dram_parameter("bd", [128, TPC * KH], f32, isOutput=False)
    bu_d = nc.declare_dram_parameter("bu", [128, TPC * KD], f32, isOutput=False)
    out_d = nc.declare_dram_parameter(
        "out", [TPC, nch, 128, KD * ck], f32, isOutput=True
    )

    with tile.TileContext(nc) as tc:
        with (
            tc.tile_pool(name="wpool", bufs=1) as wpool,
            tc.tile_pool(name="xf", bufs=2) as xfpool,
            tc.tile_pool(name="xb", bufs=2) as xbpool,
            tc.tile_pool(name="act", bufs=2) as apool,
            tc.tile_pool(name="ot", bufs=2) as opool,
            tc.tile_pool(name="psum", bufs=2, space="PSUM") as pspool,
        ):
            wd_sb = wpool.tile([128, TPC * KH * KD * 128], bf16, name="wd_sb")
            wu_sb = wpool.tile([128, TPC * KH * D], bf16, name="wu_sb")
            bd_sb = wpool.tile([128, TPC * KH], f32, name="bd_sb")
            bu_sb = wpool.tile([128, TPC * KD], f32, name="bu_sb")

            def wd_ap(t, k, h):
                base = ((t * KH + h) * KD + k) * 128
                return wd_sb[:, base: base + 128]

            def wu_ap(t, k, m):
                base = (t * KH + k) * D + m * 128
                return wu_sb[:, base: base + 128]

            xall = {}
            for t in range(TPC):
                xall[t] = [
                    xfpool.tile([128, KD * ck], f32, name=f"xall{t}_{c}",
                                tag=f"xall{c % 2}")
                    for c in range(nch)
                ]
                nc.sync.dma_start(xall[t][0][:], xt_d[t, 0])
                wslab = KD * 128
                for h in range(KH):
                    base = (t * KH + h) * wslab
                    nc.sync.dma_start(
                        wd_sb[:, base: base + wslab],
                        wd_d[:, base: base + wslab],
                    )
                for k in range(KH):
                    base = (t * KH + k) * D
                    nc.sync.dma_start(
                        wu_sb[:, base: base + D],
                        wu_d[:, base: base + D],
                    )
                if t == 0:
                    nc.sync.dma_start(bd_sb[:], bd_d[:])
                    nc.sync.dma_start(bu_sb[:], bu_d[:])
                for c in range(1, nch):
                    nc.sync.dma_start(xall[t][c][:], xt_d[t, c])

            for t in range(TPC):
                for c in range(nch):
                    xa = xall[t][c]
                    xb = xbpool.tile([128, KD * ck], bf16, name="xb", tag="xb")
                    for k in range(KD):
                        nc.vector.tensor_copy(
                            xb[:, k * ck:(k + 1) * ck],
                            xa[:, k * ck:(k + 1) * ck],
                        )
                    act = [
                        apool.tile([128, ck], bf16, name=f"act{h}", tag=f"act{h}")
                        for h in range(KH)
                    ]
                    for h in range(KH):
                        ps = pspool.tile([128, ck], f32, name=f"psd{h}", tag=f"d{h}")
                        for k in range(KD):
                            nc.tensor.matmul(
                                ps[:],
                                wd_ap(t, k, h),
                                xb[:, k * ck:(k + 1) * ck],
                                start=(k == 0),
                                stop=(k == KD - 1),
                            )
                        nc.scalar.activation(
                            act[h][:], ps[:], Silu,
                            bias=bd_sb[:, t * KH + h: t * KH + h + 1],
                            scale=1.0,
                        )
                    oall = opool.tile([128, KD * ck], f32, name="oall", tag="oall")
                    for m in range(KD):
                        psu = pspool.tile([128, ck], f32, name="psu", tag="u", bufs=3)
                        for k in range(KH):
                            nc.tensor.matmul(
                                psu[:],
                                wu_ap(t, k, m),
                                act[k][:],
                                start=(k == 0),
                                stop=(k == KH - 1),
                            )
                        nc.vector.scalar_tensor_tensor(
                            oall[:, m * ck:(m + 1) * ck], psu[:],
                            bu_sb[:, t * KD + m: t * KD + m + 1],
                            xa[:, m * ck:(m + 1) * ck],
                            op0=add, op1=add,
                        )
                    nc.sync.dma_start(out_d[t, c], oall[:])

    nc.compile()
    return nc


def kernel(x, task_id, Wd, bd, Wu, bu):
    global LAST_RESULT
    from concourse.bass_utils import run_bass_kernel_spmd
    from concourse import mybir

    fp8_mode = MODE == "fp8"
    bf16_np = mybir.dt.np(mybir.dt.bfloat16)
    fp8_np = mybir.dt.np(mybir.dt.float8e4)
    w_np = fp8_np if fp8_mode else bf16_np

    x = np.ascontiguousarray(np.asarray(x, dtype=np.float32))
    tid = np.asarray(task_id).astype(np.int64)
    Wd = np.asarray(Wd, dtype=np.float32)
    bd = np.asarray(bd, dtype=np.float32)
    Wu = np.asarray(Wu, dtype=np.float32)
    bu = np.asarray(bu, dtype=np.float32)
    B = x.shape[0]

    # --- host-side routing (the all-to-all dispatch) ---
    order = np.argsort(tid, kind="stable")
    counts = np.bincount(tid, minlength=T)
    starts = np.concatenate([[0], np.cumsum(counts)])[:T]
    cap = max(int(counts.max()), 1)
    nch = -(-cap // CK_MAX)
    ck = -(-(-(-cap // nch)) // 8) * 8  # ceil(cap/nch) rounded up to x8
    rows_per_task = nch * ck

    x_sorted = x[order]
    wscale = WSCALE if fp8_mode else 1.0

    in_maps = []
    for g in range(NCORES):
        xpad = np.zeros((TPC, rows_per_task, D), np.float32)
        for t in range(TPC):
            j = TPC * g + t
            n = counts[j]
            xpad[t, :n] = x_sorted[starts[j]: starts[j] + n]
        # [TPC, nch, 128(p), KD*ck] with col k*ck+j <-> (row j, d-tile k)
        xt_in = np.ascontiguousarray(
            xpad.reshape(TPC, nch, ck, KD, 128).transpose(0, 1, 4, 3, 2)
        ).reshape(TPC, nch, 128, KD * ck)
        sl = slice(TPC * g, TPC * g + TPC)
        # wd: [128p, TPC, KH(h), KD(k), 128(col within h-block)]
        wd_in = (
            (Wd[sl] * wscale).reshape(TPC, KD, 128, KH, 128)
            .transpose(2, 0, 3, 1, 4)
        ).reshape(128, TPC, KH * KD * 128)
        wu_in = (
            (Wu[sl] * wscale).reshape(TPC, KH, 128, D).transpose(2, 0, 1, 3)
        ).reshape(128, TPC, KH * D)
        if fp8_mode:
            w_in = np.concatenate([wd_in, wu_in], axis=2)  # [128, TPC, wtask]
            m = {
                "xt": xt_in.astype(fp8_np),
                "w": np.ascontiguousarray(
                    w_in.reshape(128, -1)
                ).astype(fp8_np),
                "bd": np.ascontiguousarray(bd[sl].reshape(TPC * KH, 128).T),
            }
        else:
            m = {
                "xt": xt_in,
                "wd": np.ascontiguousarray(wd_in.reshape(128, -1)).astype(w_np),
                "wu": np.ascontiguousarray(wu_in.reshape(128, -1)).astype(w_np),
                "bd": np.ascontiguousarray(bd[sl].reshape(TPC * KH, 128).T),
                "bu": np.ascontiguousarray(bu[sl].reshape(TPC * KD, 128).T),
            }
        in_maps.append(m)

    key = (MODE, nch, ck)
    if key not in _BUILD_CACHE:
        build = _build_fp8 if fp8_mode else _build_bf16
        _BUILD_CACHE[key] = build(nch, ck)
    nc = _BUILD_CACHE[key]

    res = run_bass_kernel_spmd(nc, in_maps, core_ids=list(range(NCORES)))
    LAST_RESULT = res

    # --- gather / unshard (inverse permutation; fp8 mode: +x +bu here) ---
    out_full = np.empty((B, D), np.float32)
    for g in range(NCORES):
        o = np.asarray(res.results[g]["out"]).astype(np.float32)
        o = o.reshape(TPC, nch, 128, KD, ck)
        o = o.transpose(0, 1, 4, 3, 2).reshape(TPC, rows_per_task, D)
        for t in range(TPC):
            j = TPC * g + t
            n = counts[j]
            rows = order[starts[j]: starts[j] + n]
            if fp8_mode:
                out_full[rows] = x[rows] + o[t, :n] + bu[j][None, :]
            else:
                out_full[rows] = o[t, :n]
    return out_full
